# revision 1
# baseline (speedup 1.0000x reference)
"""LRU (Linear Recurrent Unit) Bass kernel for Trainium2, 8 NeuronCores.

v3: fp16 + engine-balanced + software-pipelined.
  - Rotation trick: g_t = r g_{t-1} + e^{-i theta t} Bu_t; scans on Pool.
  - Issue order pipelines slab s+1's input rotation ahead of slab s's
    unrotation so the DVE has ready work while the Pool scan runs.
"""

import sys

for _p in ("/opt/trn_rl_repo", "/root/.axon_site/_ro/trn_rl_repo"):
    if _p not in sys.path:
        sys.path.append(_p)

import numpy as np

N = 256
T = 8192
BATCH = 8
NCORES = 8
S = 1024
NSLAB = T // S
F = 512
NF = S // F

_cache = {}


def _build_program():
    from concourse import bacc, tile
    from concourse import mybir

    fp32 = mybir.dt.float32
    fp16 = mybir.dt.float16
    Copy = mybir.ActivationFunctionType.Copy
    mult = mybir.AluOpType.mult
    add = mybir.AluOpType.add

    nc = bacc.Bacc(None, target_bir_lowering=False, debug=False)

    xT = nc.declare_dram_parameter("xT", [N, T], fp16, isOutput=False)
    w_bre = nc.declare_dram_parameter("w_bre", [N, N], fp16, isOutput=False)
    w_bim = nc.declare_dram_parameter("w_bim", [N, N], fp16, isOutput=False)
    w_cre = nc.declare_dram_parameter("w_cre", [N, N], fp16, isOutput=False)
    w_cimn = nc.declare_dram_parameter("w_cimn", [N, N], fp16, isOutput=False)
    cosT = nc.declare_dram_parameter("cosT", [N, T], fp16, isOutput=False)
    sinT = nc.declare_dram_parameter("sinT", [N, T], fp16, isOutput=False)
    rcol = nc.declare_dram_parameter("rcol", [N, 1], fp32, isOutput=False)
    outT = nc.declare_dram_parameter("outT", [N, T], fp16, isOutput=True)

    with tile.TileContext(nc) as tc:
        with (
            tc.tile_pool(name="const", bufs=1) as cpool,
            tc.tile_pool(name="xin", bufs=3) as xpool,
            tc.tile_pool(name="cs", bufs=3) as cspool,
            tc.tile_pool(name="bu", bufs=2) as bupool,
            tc.tile_pool(name="bt", bufs=2) as btpool,
            tc.tile_pool(name="g", bufs=2) as gpool,
            tc.tile_pool(name="h", bufs=2) as hpool,
            tc.tile_pool(name="tmp", bufs=6) as tmppool,
            tc.tile_pool(name="osb", bufs=2) as opool,
            tc.tile_pool(name="pin", bufs=1, space="PSUM") as pin,
            tc.tile_pool(name="pout", bufs=2, space="PSUM") as pout,
        ):
            # ---- slab-0 inputs first: shortens pipeline fill ----
            xt0, cs0_, sn0_ = {}, {}, {}
            for nh in range(2):
                sl = slice(nh * 128, (nh + 1) * 128)
                xt0[nh] = xpool.tile([128, S], fp16, name=f"x{nh}", tag=f"x{nh}")
                nc.sync.dma_start(out=xt0[nh][:], in_=xT[sl, 0:S])
                cs0_[nh] = cspool.tile([128, S], fp16, name=f"cos{nh}", tag=f"cos{nh}")
                nc.sync.dma_start(out=cs0_[nh][:], in_=cosT[sl, 0:S])
                sn0_[nh] = cspool.tile([128, S], fp16, name=f"sin{nh}", tag=f"sin{nh}")
                nc.sync.dma_start(out=sn0_[nh][:], in_=sinT[sl, 0:S])

            # ---- constants ----
            wb = {}
            for mat, dram in (("bre", w_bre), ("bim", w_bim),
                              ("cre", w_cre), ("cimn", w_cimn)):
                for nh in range(2):
                    wt = cpool.tile([128, N], fp16, name=f"w_{mat}{nh}", tag=f"w_{mat}{nh}")
                    nc.sync.dma_start(out=wt[:], in_=dram[nh * 128:(nh + 1) * 128, :])
                    wb[(mat, nh)] = wt
            rc, rbc = {}, {}
            for nh in range(2):
                rc[nh] = cpool.tile([128, 1], fp32, name=f"rc{nh}", tag=f"rc{nh}")
                nc.sync.dma_start(out=rc[nh][:], in_=rcol[nh * 128:(nh + 1) * 128, :])
                one_t = tmppool.tile([128, S], fp16, name=f"ones{nh}", tag=f"tmpa{nh}")
                nc.gpsimd.memset(one_t[:], 1.0)
                rb = cpool.tile([128, S], fp16, name=f"rb{nh}", tag=f"rb{nh}")
                nc.scalar.activation(rb[:], one_t[:], Copy, scale=rc[nh][:, 0:1])
                rbc[nh] = rb

            def load_slab(s):
                t0 = s * S
                xt, cs, sn = {}, {}, {}
                for nh in range(2):
                    sl = slice(nh * 128, (nh + 1) * 128)
                    xt[nh] = xpool.tile([128, S], fp16, name=f"x{nh}", tag=f"x{nh}")
                    nc.sync.dma_start(out=xt[nh][:], in_=xT[sl, t0:t0 + S])
                    cs[nh] = cspool.tile([128, S], fp16, name=f"cos{nh}", tag=f"cos{nh}")
                    nc.sync.dma_start(out=cs[nh][:], in_=cosT[sl, t0:t0 + S])
                    sn[nh] = cspool.tile([128, S], fp16, name=f"sin{nh}", tag=f"sin{nh}")
                    nc.sync.dma_start(out=sn[nh][:], in_=sinT[sl, t0:t0 + S])
                return xt, cs, sn

            def bu_slab(xt):
                bu = {}
                for mh in range(2):
                    for pl, mat in (("re", "bre"), ("im", "bim")):
                        bu_t = bupool.tile([128, S], fp16, name=f"bu_{pl}{mh}", tag=f"bu_{pl}{mh}")
                        for f in range(NF):
                            ps = pin.tile([128, F], fp32, name=f"pi{pl}{mh}", tag=f"pi{pl}{mh}")
                            for nh in range(2):
                                nc.tensor.matmul(
                                    ps[:],
                                    wb[(mat, nh)][:, mh * 128:(mh + 1) * 128],
                                    xt[nh][:, f * F:(f + 1) * F],
                                    start=(nh == 0), stop=(nh == 1),
                                )
                            nc.scalar.activation(
                                bu_t[:, f * F:(f + 1) * F], ps[:], Copy)
                        bu[(pl, mh)] = bu_t
                return bu

            def bt_slab(s, bu, cs, sn):
                bt = {}
                # slab 0 runs at f-block granularity to shorten pipeline fill
                blocks = [(0, S)] if s > 0 else [(f * F, (f + 1) * F)
                                                 for f in range(NF)]
                for mh in range(2):
                    bt_re = btpool.tile([128, S], fp16, name=f"bt_re{mh}", tag=f"bt_re{mh}")
                    bt_im = btpool.tile([128, S], fp16, name=f"bt_im{mh}", tag=f"bt_im{mh}")
                    bt[("re", mh)] = bt_re
                    bt[("im", mh)] = bt_im
                for lo, hi in blocks:
                    for mh in range(2):
                        w = hi - lo
                        a = tmppool.tile([128, w], fp16, name=f"bta{mh}", tag=f"tmpa{mh}")
                        b = tmppool.tile([128, w], fp16, name=f"btb{mh}", tag=f"tmpb{mh}")
                        bt_re, bt_im = bt[("re", mh)], bt[("im", mh)]
                        nc.vector.tensor_mul(a[:], cs[mh][:, lo:hi], bu[("re", mh)][:, lo:hi])
                        nc.vector.tensor_mul(b[:], sn[mh][:, lo:hi], bu[("im", mh)][:, lo:hi])
                        nc.vector.tensor_add(bt_re[:, lo:hi], a[:], b[:])
                        if mh == 0 and (s % 2 == 1):
                            nc.gpsimd.tensor_mul(a[:], cs[mh][:, lo:hi], bu[("im", mh)][:, lo:hi])
                        else:
                            nc.vector.tensor_mul(a[:], cs[mh][:, lo:hi], bu[("im", mh)][:, lo:hi])
                        nc.vector.tensor_mul(b[:], sn[mh][:, lo:hi], bu[("re", mh)][:, lo:hi])
                        nc.vector.tensor_sub(bt_im[:, lo:hi], a[:], b[:])
                return bt

            # ---- prologue: slab 0 through its rotation ----
            xt, cs0, sn0 = xt0, cs0_, sn0_
            bu = bu_slab(xt)
            bt = bt_slab(0, bu, cs0, sn0)
            cs_cur, sn_cur = cs0, sn0
            g_prev = {}

            for s in range(NSLAB):
                # ---- scans for slab s on Pool ----
                g = {}
                if s == 0:
                    for fb in range(NF):
                        lo, hi = fb * F, (fb + 1) * F
                        for pl in ("re", "im"):
                            for mh in range(2):
                                if fb == 0:
                                    g_t = gpool.tile([128, S], fp16, name=f"g_{pl}{mh}", tag=f"g_{pl}{mh}")
                                    g[(pl, mh)] = g_t
                                    init = 0.0
                                else:
                                    g_t = g[(pl, mh)]
                                    init = g_t[:, lo - 1:lo]
                                nc.vector.tensor_tensor_scan(
                                    g_t[:, lo:hi], rbc[mh][:, lo:hi],
                                    bt[(pl, mh)][:, lo:hi],
                                    init, mult, add,
                                )
                else:
                    for pl in ("re", "im"):
                        for mh in range(2):
                            g_t = gpool.tile([128, S], fp16, name=f"g_{pl}{mh}", tag=f"g_{pl}{mh}")
                            init = g_prev[(pl, mh)][:, S - 1:S]
                            nc.vector.tensor_tensor_scan(
                                g_t[:], rbc[mh][:], bt[(pl, mh)][:],
                                init, mult, add,
                            )
                            g[(pl, mh)] = g_t
                g_prev = g

                # ---- pipeline: slab s+1 input side (keeps DVE busy) ----
                if s + 1 < NSLAB:
                    xt, cs_n, sn_n = load_slab(s + 1)
                    bu = bu_slab(xt)
                    bt = bt_slab(s + 1, bu, cs_n, sn_n)
                else:
                    cs_n = sn_n = None

                # ---- unrotate + project; last slab at f-block granularity
                # to overlap the drain tail ----
                t0 = s * S
                h = {}
                o_t = {}
                for mh in range(2):
                    h[("re", mh)] = hpool.tile([128, S], fp16, name=f"h_re{mh}", tag=f"h_re{mh}")
                    h[("im", mh)] = hpool.tile([128, S], fp16, name=f"h_im{mh}", tag=f"h_im{mh}")
                for mh_o in range(2):
                    o_t[mh_o] = opool.tile([128, S], fp16, name=f"o{mh_o}", tag=f"o{mh_o}")
                hblocks = [(0, S)] if s < NSLAB - 1 else [
                    (f * F, (f + 1) * F) for f in range(NF)]
                for lo, hi in hblocks:
                    w = hi - lo
                    for mh in range(2):
                        a = tmppool.tile([128, w], fp16, name=f"ha{mh}", tag=f"tmpa{mh}")
                        b = tmppool.tile([128, w], fp16, name=f"hb{mh}", tag=f"tmpb{mh}")
                        h_re, h_im = h[("re", mh)], h[("im", mh)]
                        nc.vector.tensor_mul(a[:], cs_cur[mh][:, lo:hi], g[("re", mh)][:, lo:hi])
                        nc.vector.tensor_mul(b[:], sn_cur[mh][:, lo:hi], g[("im", mh)][:, lo:hi])
                        nc.vector.tensor_sub(h_re[:, lo:hi], a[:], b[:])
                        nc.gpsimd.tensor_mul(a[:], sn_cur[mh][:, lo:hi], g[("re", mh)][:, lo:hi])
                        nc.vector.tensor_mul(b[:], cs_cur[mh][:, lo:hi], g[("im", mh)][:, lo:hi])
                        nc.vector.tensor_add(h_im[:, lo:hi], a[:], b[:])
                    for fo in range(lo // F, hi // F):
                        flo, fhi = fo * F, (fo + 1) * F
                        for mh_o in range(2):
                            ps = pout.tile([128, F], fp32, name=f"po{mh_o}", tag=f"po{mh_o}")
                            k = 0
                            for pl, mat in (("re", "cre"), ("im", "cimn")):
                                for mh in range(2):
                                    nc.tensor.matmul(
                                        ps[:],
                                        wb[(mat, mh)][:, mh_o * 128:(mh_o + 1) * 128],
                                        h[(pl, mh)][:, flo:fhi],
                                        start=(k == 0), stop=(k == 3),
                                    )
                                    k += 1
                            nc.scalar.activation(o_t[mh_o][:, flo:fhi], ps[:], Copy)
                for mh_o in range(2):
                    nc.sync.dma_start(
                        out=outT[mh_o * 128:(mh_o + 1) * 128, t0:t0 + S],
                        in_=o_t[mh_o][:],
                    )
                cs_cur, sn_cur = cs_n, sn_n

    nc.compile()
    return nc


def _host_prep(x, nu_log, theta_log, gamma_log, B_re, B_im, C_re, C_im):
    f64 = np.float64
    r = np.exp(-np.exp(nu_log.astype(f64)))
    theta = np.exp(theta_log.astype(f64))
    gamma = np.exp(gamma_log.astype(f64))
    Bn_re = (B_re.astype(f64) * gamma[:, None])
    Bn_im = (B_im.astype(f64) * gamma[:, None])
    t = np.arange(T, dtype=f64)
    ang = theta[:, None] * t[None, :]
    shared = {
        "w_bre": np.ascontiguousarray(Bn_re.T).astype(np.float16),
        "w_bim": np.ascontiguousarray(Bn_im.T).astype(np.float16),
        "w_cre": np.ascontiguousarray(C_re.T).astype(np.float16),
        "w_cimn": np.ascontiguousarray(-C_im.T).astype(np.float16),
        "cosT": np.cos(ang).astype(np.float16),
        "sinT": np.sin(ang).astype(np.float16),
        "rcol": r[:, None].astype(np.float32),
    }
    xTs = [np.ascontiguousarray(x[b].T).astype(np.float16) for b in range(BATCH)]
    return shared, xTs


def kernel(x, nu_log, theta_log, gamma_log, B_re, B_im, C_re, C_im,
           _want_trace=False):
    from concourse import bass_utils

    x = np.asarray(x)
    nu_log = np.asarray(nu_log)
    theta_log = np.asarray(theta_log)
    gamma_log = np.asarray(gamma_log)
    B_re, B_im = np.asarray(B_re), np.asarray(B_im)
    C_re, C_im = np.asarray(C_re), np.asarray(C_im)

    if "nc" not in _cache:
        _cache["nc"] = _build_program()
    nc = _cache["nc"]

    shared, xTs = _host_prep(x, nu_log, theta_log, gamma_log,
                             B_re, B_im, C_re, C_im)
    in_maps = [dict(shared, xT=xTs[i]) for i in range(NCORES)]
    import os
    os.environ["BASS_NEVER_TRACE"] = "1"
    res = bass_utils.run_bass_kernel_spmd(
        nc, in_maps, core_ids=list(range(NCORES)), trace=False,
    )
    _cache["last_result"] = res
    out = np.stack([res.results[i]["outT"].T for i in range(NCORES)])
    return out.astype(np.float32)


if __name__ == "__main__":
    rng = np.random.default_rng(0)
    ins = {
        "x": rng.standard_normal((BATCH, T, N), dtype=np.float32),
        "nu_log": rng.standard_normal(N).astype(np.float32),
        "theta_log": rng.standard_normal(N).astype(np.float32),
        "gamma_log": rng.standard_normal(N).astype(np.float32),
        "B_re": rng.standard_normal((N, N), dtype=np.float32) * 0.04,
        "B_im": rng.standard_normal((N, N), dtype=np.float32) * 0.04,
        "C_re": rng.standard_normal((N, N), dtype=np.float32) * 0.06,
        "C_im": rng.standard_normal((N, N), dtype=np.float32) * 0.06,
    }
    out = kernel(**ins)
    print("out", out.shape, out.dtype, np.abs(out).max())

